# revision 52
# baseline (speedup 1.0000x reference)
"""Sharded sparse (windowed) attention for TRN2 — 8 NeuronCores, head-parallel.

Reference computation (B=4, N=197, C=2048, H=32 heads, hd=64, window=8):
    qkv = x @ qkv_w.T -> split q,k,v per head
    attn = softmax(mask_weight * (q@k.T) * hd^-0.5  with off-band -inf)
    out  = (attn @ v) per head, concat heads, @ proj_w.T + proj_b
Sharding: 4 heads per core (tensor parallel). Each core computes its heads'
qkv projection, windowed attention, and a partial of the output projection
(contraction over its 256 head-dims). Host sums the 8 partials + bias.

On-device layout is transposed (feature dim on partitions, tokens on the
free axis) until the output projection, which flips to (tokens, features):
    xT (2048, 788)  qkT (512, 788)  v (tokens, 256)  E=(j,i)  out (788, 2048)

Structure (measured ~83.5us HW exec, rel err 7.2e-4; v0 baseline ~108us):
  - xw input via few large DMAs split across BOTH HWDGE rings (nc.scalar +
    nc.sync) so descriptor generation overlaps; issued before pw/masks
  - 12 f32 warm-up matmuls bridge the ~13us preamble/IRAM-fetch window so
    HAM un-throttles the PE clock before phase 1's data lands
  - phase 1 (q,k) ni-inner: 8 PSUM tiles across all 8 banks; each xw chunk
    is consumed by 8 back-to-back matmuls, matching the DMA arrival rate
  - score blocks DISJOINT in i (no PSUM accumulation): blk0 i<120,
    blk1 i>=120 (its j rows [69,197) cover the window for those queries)
  - each head has its OWN score PSUM tile: the two heads of a pair contract
    on different PE row groups (partitions 0:64 / 64:128), and
    different-row-group matmuls draining into the SAME PSUM bank abort the
    device; adjacent score matmuls alternate row group AND bank, so they
    execute concurrently in the array
  - chain ops run once per head PAIR on [128, 396] tiles (mask-add merges
    the two score tiles into one wide SBUF tile; mask-mult at 2x DVE rate;
    one exp); packed zt [65, 396] is safe (AV matmuls are full-array)
  - only the softmax denominator row is staged to SBUF (the custom-DVE
    reciprocal reads garbage from PSUM on HW); z itself is normalized
    straight out of PSUM; mask pad columns are (M=0, A=0) so e=1 there and
    the packed reciprocal stays finite
  - output projection is YT-stationary: out[tok, 2048] = YT_chunk^T @ pw
    with N=512 streams (LDWEIGHTS hidden), accumulated over 2 k-chunks,
    evacuated into a [tok, 2048] fp16 tile, DMA per (batch, chunk, 1024
    cols); per-batch proj interleaved into the next batch's chain slots
  - v for batches 1..3 and the previous batch's proj are PE filler inside
    the chain slots; keep-warm dummy matmuls (overwritten by the real
    start=True matmuls) prevent HAM re-throttling in the late phase
  - out partials fp16 (host accumulates in f32)
"""

import numpy as np

B = 4
N = 197
C = 2048
H = 32
HD = 64
WIN = 8
NCORES = 8
HPC = H // NCORES          # heads per core
CPC = HPC * HD             # head-dims per core (256)
T = B * N                  # 788 tokens
TP = T + 2                 # padded qkT width (block-1 rhs reads col 788)
KC = C // 128              # 16 contraction chunks
SCALE = HD ** -0.5
NEG = -200.0               # additive off-band mask (exp underflows to 0)

# banded blocks of ST[j, i]: (j0, jh, i0, iw, packed column offset)
# blocks are 128 j-rows and DISJOINT in i: block 0 handles queries i<120
# (window j<=127 fits its rows), block 1 handles i>=120 (window j>=112 fits
# its rows j in [69,197)). No overlap -> no PSUM accumulation between them.
BLOCKS = [(0, 128, 0, 120, 0), (69, 128, 120, 78, 120)]
SW = 198                   # packed score-tile width per head (120 + 78)
SW2 = 2 * SW               # two heads side by side
NP = 198                   # padded zt width
TOKCH = [(0, 128), (69, 128)]          # per-batch token chunks (v rows)
QKCH = [(0, 394), (394, 394)]          # qk token halves
PROJCH = [(0, 128), (128, 69)]         # proj token chunks (disjoint rows)
NS = 4                     # proj output column slices (2048 / 512)
XGROUPS = [[0], [1], [2, 3], [4, 5, 6, 7], [8, 9, 10, 11], [12, 13, 14, 15]]

# Measured configs (HW exec / end-to-end relative error):
#   fp16/fp16: ~83.5us / 7.2e-4   bf16/bf16: ~86us / 5.8e-3
DT_BIG = "fp16"            # qkv + proj matmul operand dtype: fp16|f32r|bf16
DT_ATT = "fp16"            # attention matmul operand dtype:  fp16|f32r|bf16

_compiled = {}


def _dt(mybir, name):
    return {"f32r": mybir.dt.float32r, "bf16": mybir.dt.bfloat16,
            "fp16": mybir.dt.float16}[name]


def _build_program(dt_big, dt_att):
    import concourse.mybir as mybir
    import concourse.tile as tile
    from concourse import bacc

    F32 = mybir.dt.float32
    DTB = _dt(mybir, dt_big)
    DTA = _dt(mybir, dt_att)

    nc = bacc.Bacc("TRN2", target_bir_lowering=False, debug=False)

    # xw = [xT | wqkT | wvT] packed on the 2048-row contraction axis
    XWW = T + 2 * CPC + CPC
    xw = nc.dram_tensor("xw", [C, XWW], DTB, kind="ExternalInput")
    pw = nc.dram_tensor("pw", [CPC, C], DTB, kind="ExternalInput")
    maskA = nc.dram_tensor("maskA", [128, SW2], DTA, kind="ExternalInput")
    maskM = nc.dram_tensor("maskM", [128, SW2], DTA, kind="ExternalInput")
    out_d = nc.dram_tensor("out", [T, C], DTB, kind="ExternalOutput")

    with tile.TileContext(nc) as tc:
        with (
            tc.tile_pool(name="persist", bufs=1) as per,
            tc.tile_pool(name="work", bufs=8) as wk,
            tc.tile_pool(name="ps", bufs=2, space="PSUM") as pp,
        ):
            # ---- input loads first: xw in few large DMAs (group 0 split so
            # the first matmul chunk lands fast, weights before activations)
            xwt = []            # kc -> [128, XWW] view
            for gi, g in enumerate(XGROUPS):
                gw = len(g) * XWW
                gt = per.tile([128, gw], DTB, tag=f"xwg{g[0]}")
                src = xw[g[0] * 128:(g[0] + len(g)) * 128, :]
                # alternate the two HWDGE rings (ACT + sync queues) so the
                # ~0.8us-per-dma_start descriptor generation runs in
                # parallel and the first bytes land sooner
                eng = nc.scalar if gi % 2 == 0 else nc.sync
                eng.dma_start(
                    out=gt.rearrange("p (n c) -> p n c", n=len(g)),
                    in_=src.rearrange("(n p) c -> p n c", p=128))
                for i, kc in enumerate(g):
                    xwt.append(gt[:, i * XWW:(i + 1) * XWW])
            xt = [t[:, 0:T] for t in xwt]
            wqk_t = [t[:, T:T + 2 * CPC] for t in xwt]
            wv_t = [t[:, T + 2 * CPC:XWW] for t in xwt]

            # ---- small constants after xw (needed only mid-kernel) ----
            # masks/pw ride the scalar ring: keeps the sync ring free to
            # issue the remaining xw groups ~1.6us earlier (ACT itself is
            # idle until the attention phase, so the issue time is free)
            mA = per.tile([128, SW2], DTA, tag="mA")
            nc.scalar.dma_start(out=mA, in_=maskA[:, :])
            mM = per.tile([128, SW2], DTA, tag="mM")
            nc.scalar.dma_start(out=mM, in_=maskM[:, :])
            pw_t = []
            for k2 in range(2):
                t = per.tile([128, C], DTB, tag=f"pw{k2}")
                nc.scalar.dma_start(out=t, in_=pw[k2 * 128:(k2 + 1) * 128, :])
                pw_t.append(t)
            onecol = per.tile([128, 1], F32, tag="onecol")
            nc.vector.memset(onecol, 1.0)
            zpad = per.tile([128, TP - T], F32, tag="zpad")
            nc.vector.memset(zpad, 0.0)

            # warm-up matmuls on memset data (no DMA dependency): keeps the
            # PE busy while the first x/w chunks stream in, so HAM
            # un-throttles the clock early.
            wusrc = per.tile([128, 256], F32, tag="wusrc")
            nc.vector.memset(wusrc, 1.0)
            wusrc16 = per.tile([128, 128], DTA, tag="wusrc16")
            nc.vector.memset(wusrc16, 1.0)
            # 12 f32 N=256 matmuls ~= 10us at the cold clock: the PE stays
            # busy through the IRAM-fetch/preamble window (first input bytes
            # land ~12us in) so HAM un-throttles early and phase 1 starts
            # warm the moment its data arrives.
            wu = pp.tile([128, 256], F32, tag="mm", name="wu", bufs=3)
            for _ in range(12):
                nc.tensor.matmul(out=wu, lhsT=wusrc[:, 0:128], rhs=wusrc,
                                 start=True, stop=True)

            # ---- phase 1: q,k projection (weight-stationary) ----
            # qkT[mc] rows: mc 0,1 = q (heads 0,1 / 2,3); mc 2,3 = k
            # mc alternates innermost so consecutive matmuls load different
            # weights (LDWEIGHTS overlaps via the background weight buffer).
            qkT = []
            for mc4 in range(4):
                t = per.tile([128, TP], DTA, tag=f"qkT{mc4}", name=f"qkT{mc4}")
                nc.vector.tensor_copy(out=t[:, T:TP], in_=zpad)
                qkT.append(t)
            # ni inner: 8 qkps tiles live across all 8 banks, and each
            # xw chunk is consumed by 8 back-to-back matmuls -- the sweep
            # rate then matches the input-DMA arrival rate (no mid-phase
            # PE stall -> no HAM re-throttle).
            gtag = {(0, 0): "mm", (0, 1): "mm", (1, 0): "mm", (1, 1): "st",
                    (2, 0): "st", (2, 1): "st", (3, 0): "zt", (3, 1): "zt"}
            qps = {}
            for mc in range(4):
                for ni, (n0, nw) in enumerate(QKCH):
                    qps[(mc, ni)] = pp.tile(
                        [128, nw], F32, tag=gtag[(mc, ni)],
                        name=f"qkps{mc}_{ni}", bufs=3 if gtag[(mc, ni)] != "zt" else 2)
            for kc in range(KC):
                for ni, (n0, nw) in enumerate(QKCH):
                    for mc in range(4):
                        nc.tensor.matmul(
                            out=qps[(mc, ni)],
                            lhsT=wqk_t[kc][:, mc * 128:(mc + 1) * 128],
                            rhs=xt[kc][:, n0:n0 + nw],
                            start=(kc == 0), stop=(kc == KC - 1),
                        )
            for ni, (n0, nw) in enumerate(QKCH):
                for mc in range(4):
                    if (mc + ni) % 2 == 0:
                        nc.scalar.copy(out=qkT[mc][:, n0:n0 + nw],
                                       in_=qps[(mc, ni)])
                    else:
                        nc.vector.tensor_copy(out=qkT[mc][:, n0:n0 + nw],
                                              in_=qps[(mc, ni)])

            # ---- phase 2: v projection (x-stationary: v in (tokens, dims)) ----
            vone = {}  # (b, jc) -> [th, 4, 65] tile (per-head v cols + ones col)

            def emit_v(b, jc):
                t0, th = TOKCH[jc]
                vps = pp.tile([th, CPC], F32, tag="mm", name=f"vps{b}_{jc}", bufs=3)
                for kc in range(KC):
                    nc.tensor.matmul(
                        out=vps,
                        lhsT=xt[kc][:, b * N + t0: b * N + t0 + th],
                        rhs=wv_t[kc],
                        start=(kc == 0), stop=(kc == KC - 1),
                    )
                vt = per.tile([th, HPC, HD + 1], DTA, tag=f"vone{b}_{jc}",
                              name=f"vone{b}_{jc}")
                nc.vector.tensor_copy(
                    out=vt[:, :, 0:HD],
                    in_=vps.rearrange("t (h d) -> t h d", h=HPC))
                nc.vector.tensor_copy(
                    out=vt[:, :, HD],
                    in_=onecol[:th, 0:1].to_broadcast((th, HPC)))
                vone[(b, jc)] = vt



            # ---- phase 3 + 4: windowed attention, proj interleaved ----
            # per (b,h): ST packed [128, 272]; logits = (ST + A) * M
            # E = exp(logits); zT' = [v|1]^T @ E  (row HD = softmax denom)
            # After batch b completes: project its 197 columns (spread over
            # the next batch's head slots).
            YT = [per.tile([128, T], DTB, tag=f"YT{k2}", name=f"YT{k2}")
                  for k2 in range(2)]

            def attn_s(b, p):
                """score matmuls for head pair (2p, 2p+1) -> two st tiles.
                Each head gets its OWN tile: the two heads contract on
                different PE row groups (partitions 0:64 vs 64:128), and
                different-row-group matmuls draining into the same PSUM
                bank abort the device."""
                st = [pp.tile([128, SW], F32, tag="st",
                              name=f"st{b}_{p}_{s2}", bufs=3)
                      for s2 in range(2)]
                if b >= 2:
                    # keep-warm dummy: HAM re-throttles the PE clock to
                    # 1.2GHz when the late phase goes sparse; this is
                    # overwritten by the real start=True matmuls below
                    nc.tensor.matmul(out=st[0][:, 0:64],
                                     lhsT=wusrc16, rhs=wusrc16[:, 0:64],
                                     start=True, stop=True)
                # blk outer / head inner: adjacent matmuls use different PE
                # row groups AND different PSUM banks, so they execute
                # concurrently in the array (per-subarray concurrency)
                for (j0, jh, i0, iw, c0) in BLOCKS:
                    for s2 in range(2):
                        qTh = qkT[p][64 * s2:64 * s2 + 64, :]
                        kTh = qkT[2 + p][64 * s2:64 * s2 + 64, :]
                        nc.tensor.matmul(
                            out=st[s2][0:jh, c0:c0 + iw],
                            lhsT=kTh[:, b * N + j0: b * N + j0 + jh],
                            rhs=qTh[:, b * N + i0: b * N + i0 + iw],
                            start=True, stop=True,
                        )
                return st

            def attn_rest(b, p, st):
                """mask, exp, AV, normalize for head pair (2p, 2p+1).
                The per-head PSUM score tiles merge into one wide SBUF tile
                at the first (1x-rate PSUM-read) op; everything downstream
                runs once per pair. The AV matmuls are full-array, so the
                packed zt bank is safe."""
                sa = wk.tile([128, SW2], DTA, tag="sa")
                for s2 in range(2):
                    nc.vector.tensor_tensor(
                        sa[:, s2 * SW:(s2 + 1) * SW], st[s2], mA[:, 0:SW],
                        mybir.AluOpType.add)
                sm = wk.tile([128, SW2], DTA, tag="sm")
                nc.vector.tensor_tensor(sm, sa, mM, mybir.AluOpType.mult)
                e = wk.tile([128, SW2], DTA, tag="e")
                nc.scalar.activation(
                    out=e, in_=sm, func=mybir.ActivationFunctionType.Exp)
                zt = pp.tile([HD + 1, SW2], F32, tag="zt", name=f"zt{b}_{p}")
                first = True
                for s2 in range(2):
                    for blk, (j0, jh, i0, iw, c0) in enumerate(BLOCKS):
                        nc.tensor.matmul(
                            out=zt[:, s2 * SW + i0: s2 * SW + i0 + iw],
                            lhsT=vone[(b, blk)][0:jh, 2 * p + s2, :],
                            rhs=e[0:jh, s2 * SW + c0: s2 * SW + c0 + iw],
                            start=first, stop=(s2 == 1 and blk == 1),
                        )
                        first = False
                # stage only the denominator row to SBUF (the custom-DVE
                # reciprocal cannot read PSUM on HW); z is normalized
                # straight out of PSUM. Pad columns hold den=128 (mask
                # makes e=1 there) so the reciprocal stays finite.
                dent = wk.tile([1, SW2], F32, tag="dent")
                nc.scalar.copy(out=dent, in_=zt[HD:HD + 1, :])
                rrow = wk.tile([1, SW2], F32, tag="rrow")
                nc.vector.reciprocal_approx_fast(out=rrow, in_=dent)
                rb = wk.tile([64, SW2], F32, tag="rb")
                nc.gpsimd.partition_broadcast(rb, rrow)
                for s2 in range(2):
                    nc.vector.tensor_tensor(
                        YT[p][s2 * 64:(s2 + 1) * 64, b * N:(b + 1) * N],
                        zt[0:HD, s2 * SW:s2 * SW + N],
                        rb[:, s2 * SW:s2 * SW + N], mybir.AluOpType.mult)

            obt = {}
            for b2 in range(B):
                for jc2 in range(2):
                    obt[(b2, jc2)] = per.tile(
                        [PROJCH[jc2][1], C], DTB, tag=f"ob{jc2}",
                        name=f"ob{b2}_{jc2}", bufs=2)

            pj_pend = {}

            def proj_units(b, units, tag, k2s=(0, 1)):
                # units arrive as (jc, ns) pairs with even ns: each call
                # handles (ns, ns+1) with k2 outer so the YT weight tile is
                # reused by two consecutive matmuls. k2s allows splitting
                # the accumulation across two call sites (tail shortening:
                # the k2=0 half only needs YT[0] = head pair 0).
                for (jc, ns) in units:
                    t0, th = PROJCH[jc]
                    tsl = slice(b * N + t0, b * N + t0 + th)
                    if 0 in k2s:
                        pps = [pp.tile([th, 512], F32, tag="mm",
                                       name=f"pj{tag}_{jc}_{ns + u}", bufs=3)
                               for u in range(2)]
                        pj_pend[(b, jc, ns)] = pps
                        if (tag == "p3" or b >= 2) and (jc, ns) == units[0]:
                            nc.tensor.matmul(out=pps[0][0:th, 0:64],
                                             lhsT=wusrc16[:, 0:th],
                                             rhs=wusrc16[:, 0:64],
                                             start=True, stop=True)
                    else:
                        pps = pj_pend.pop((b, jc, ns))
                    for k2 in k2s:
                        for u in range(2):
                            nc.tensor.matmul(
                                out=pps[u],
                                lhsT=YT[k2][:, tsl],
                                rhs=pw_t[k2][:, (ns + u) * 512:(ns + u + 1) * 512],
                                start=(k2 == 0), stop=(k2 == 1),
                            )
                    if 1 not in k2s:
                        continue
                    ob = obt[(b, jc)]
                    for u in range(2):
                        # 3:1 toward ScalarE mid-kernel (VectorE carries the
                        # chain ops); 1:1 for the final batch where the
                        # evacuations are the critical path to the last DMA
                        if u == 1 and (jc == 1 or tag == "p3"):
                            nc.vector.tensor_copy(
                                out=ob[:, (ns + u) * 512:(ns + u + 1) * 512],
                                in_=pps[u])
                        else:
                            nc.scalar.copy(
                                out=ob[:, (ns + u) * 512:(ns + u + 1) * 512],
                                in_=pps[u])
                    csl = slice(ns * 512, (ns + 2) * 512)
                    nc.sync.dma_start(out=out_d[tsl, csl], in_=ob[:, csl])

            # software-pipeline: emit S matmuls one (b,h) ahead so the PE has
            # independent work while the previous chain's DVE/ACT stages run.
            # Fillers per head slot keep the PE dense: v-projections for the
            # next batches, and the previous batch's proj M-chunks.
            PUNITS = [(jc, ns) for jc in range(2) for ns in range(0, NS, 2)]
            bps = [(b, p) for b in range(B) for p in range(2)]
            vfill = [(1, 0), (1, 1), (2, 0), (2, 1), (3, 0), (3, 1)]
            # first chain's scores before the v matmuls: its mask/exp stages
            # then overlap the v projection on the PE
            sts = {bps[0]: attn_s(*bps[0])}
            emit_v(0, 0)
            emit_v(0, 1)
            for idx, (b, p) in enumerate(bps):
                if idx + 1 < len(bps):
                    sts[bps[idx + 1]] = attn_s(*bps[idx + 1])
                attn_rest(b, p, sts.pop((b, p)))
                if vfill:
                    emit_v(*vfill.pop(0))
                if b >= 1:
                    proj_units(b - 1, PUNITS[p * 2:(p + 1) * 2], f"p{b-1}")
            proj_units(3, PUNITS, "p3")

    nc.compile()
    return nc


def _host_masks(np_att):
    i = np.arange(N)[:, None]
    j = np.arange(N)[None, :]
    d = np.abs(i - j).astype(np.float32)
    in_win = (j >= i - WIN) & (j < i + WIN)
    m = np.where(in_win, (WIN - d / 2.0) / WIN, 0.0).astype(np.float32)
    # transposed (j on rows): logits[j,i] = (ST[j,i] + A[j,i]) * M[j,i]
    multT = np.where(in_win, m * SCALE, 1.0).astype(np.float32).T
    addT = np.where(in_win, 0.0, NEG).astype(np.float32).T
    # pack the two banded blocks side by side into [128, SW] tiles
    mult = np.zeros((128, SW), dtype=np.float32)
    addm = np.zeros((128, SW), dtype=np.float32)
    for blk, (j0, jh, i0, iw, c0) in enumerate(BLOCKS):
        iw_r = min(iw, N - i0)  # data columns (rest stays pad)
        mult[0:jh, c0:c0 + iw_r] = multT[j0:j0 + jh, i0:i0 + iw_r]
        addm[0:jh, c0:c0 + iw_r] = addT[j0:j0 + jh, i0:i0 + iw_r]
    # duplicate for the two heads packed side by side; pad cells stay
    # (M=0, A=0) so e = exp(0) = 1 and denominators remain finite.
    return (np.tile(addm, (1, 2)).astype(np_att),
            np.tile(mult, (1, 2)).astype(np_att))


def _np_dt(name):
    if name == "bf16":
        import ml_dtypes
        return ml_dtypes.bfloat16
    if name == "fp16":
        return np.float16
    return np.float32


def _make_in_maps(x, qkv_w, proj_w):
    npb = _np_dt(DT_BIG)
    npa = _np_dt(DT_ATT)
    xT = x.reshape(T, C).T
    addm, mult = _host_masks(npa)
    in_maps = []
    for d in range(NCORES):
        r = slice(d * CPC, (d + 1) * CPC)
        wqk_d = np.concatenate(
            [qkv_w[r, :], qkv_w[C + d * CPC: C + (d + 1) * CPC, :]], axis=0).T
        wv_d = qkv_w[2 * C + d * CPC: 2 * C + (d + 1) * CPC, :].T
        xw_d = np.ascontiguousarray(
            np.concatenate([xT, wqk_d, wv_d], axis=1)).astype(npb)
        pw_d = np.ascontiguousarray(proj_w[:, r].T).astype(npb)
        in_maps.append({"xw": xw_d, "pw": pw_d, "maskA": addm, "maskM": mult})
    return in_maps


def kernel(x, qkv_w, proj_w, proj_b):
    from concourse.bass_utils import run_bass_kernel_spmd

    key = (DT_BIG, DT_ATT)
    if key not in _compiled:
        _compiled[key] = _build_program(*key)
    nc = _compiled[key]

    x = np.asarray(x, dtype=np.float32)
    qkv_w = np.asarray(qkv_w, dtype=np.float32)
    proj_w = np.asarray(proj_w, dtype=np.float32)
    proj_b = np.asarray(proj_b, dtype=np.float32)

    in_maps = _make_in_maps(x, qkv_w, proj_w)
    res = run_bass_kernel_spmd(nc, in_maps, core_ids=list(range(NCORES)))
    acc = np.zeros((T, C), dtype=np.float32)
    for r in res.results:
        acc += r["out"].astype(np.float32)
    out = acc + proj_b[None, :]
    return np.ascontiguousarray(out).reshape(B, N, C)


# revision 53
# speedup vs baseline: 1.0217x; 1.0217x over previous
"""Sharded sparse (windowed) attention for TRN2 — 8 NeuronCores, head-parallel.

Reference computation (B=4, N=197, C=2048, H=32 heads, hd=64, window=8):
    qkv = x @ qkv_w.T -> split q,k,v per head
    attn = softmax(mask_weight * (q@k.T) * hd^-0.5  with off-band -inf)
    out  = (attn @ v) per head, concat heads, @ proj_w.T + proj_b
Sharding: 4 heads per core (tensor parallel). Each core computes its heads'
qkv projection, windowed attention, and a partial of the output projection
(contraction over its 256 head-dims). Host sums the 8 partials + bias.

On-device layout is transposed (feature dim on partitions, tokens on the
free axis) until the output projection, which flips to (tokens, features):
    xT (2048, 788)  qkT (512, 788)  v (tokens, 256)  E=(j,i)  out (788, 2048)

Structure (measured ~83.5us HW exec, rel err 7.2e-4; v0 baseline ~108us):
  - xw input via few large DMAs split across BOTH HWDGE rings (nc.scalar +
    nc.sync) so descriptor generation overlaps; issued before pw/masks
  - 12 f32 warm-up matmuls bridge the ~13us preamble/IRAM-fetch window so
    HAM un-throttles the PE clock before phase 1's data lands
  - phase 1 (q,k) ni-inner: 8 PSUM tiles across all 8 banks; each xw chunk
    is consumed by 8 back-to-back matmuls, matching the DMA arrival rate
  - score blocks DISJOINT in i (no PSUM accumulation): blk0 i<120,
    blk1 i>=120 (its j rows [69,197) cover the window for those queries)
  - each head has its OWN score PSUM tile: the two heads of a pair contract
    on different PE row groups (partitions 0:64 / 64:128), and
    different-row-group matmuls draining into the SAME PSUM bank abort the
    device; adjacent score matmuls alternate row group AND bank, so they
    execute concurrently in the array
  - chain ops run once per head PAIR on [128, 396] tiles (mask-add merges
    the two score tiles into one wide SBUF tile; mask-mult at 2x DVE rate;
    one exp); packed zt [65, 396] is safe (AV matmuls are full-array)
  - only the softmax denominator row is staged to SBUF (the custom-DVE
    reciprocal reads garbage from PSUM on HW); z itself is normalized
    straight out of PSUM; mask pad columns are (M=0, A=0) so e=1 there and
    the packed reciprocal stays finite
  - output projection is YT-stationary: out[tok, 2048] = YT_chunk^T @ pw
    with N=512 streams (LDWEIGHTS hidden), accumulated over 2 k-chunks,
    evacuated into a [tok, 2048] fp16 tile, DMA per (batch, chunk, 1024
    cols); per-batch proj interleaved into the next batch's chain slots
  - v for batches 1..3 and the previous batch's proj are PE filler inside
    the chain slots; keep-warm dummy matmuls (overwritten by the real
    start=True matmuls) prevent HAM re-throttling in the late phase
  - out partials fp16 (host accumulates in f32)
"""

import numpy as np

B = 4
N = 197
C = 2048
H = 32
HD = 64
WIN = 8
NCORES = 8
HPC = H // NCORES          # heads per core
CPC = HPC * HD             # head-dims per core (256)
T = B * N                  # 788 tokens
TP = T + 2                 # padded qkT width (block-1 rhs reads col 788)
KC = C // 128              # 16 contraction chunks
SCALE = HD ** -0.5
NEG = -200.0               # additive off-band mask (exp underflows to 0)

# banded blocks of ST[j, i]: (j0, jh, i0, iw, packed column offset)
# blocks are 128 j-rows and DISJOINT in i: block 0 handles queries i<120
# (window j<=127 fits its rows), block 1 handles i>=120 (window j>=112 fits
# its rows j in [69,197)). No overlap -> no PSUM accumulation between them.
BLOCKS = [(0, 128, 0, 120, 0), (69, 128, 120, 78, 120)]
SW = 198                   # packed score-tile width per head (120 + 78)
SW2 = 2 * SW               # two heads side by side
NP = 198                   # padded zt width
TOKCH = [(0, 128), (69, 128)]          # per-batch token chunks (v rows)
QKCH = [(0, 394), (394, 394)]          # qk token halves
PROJCH = [(0, 128), (128, 69)]         # proj token chunks (disjoint rows)
NS = 4                     # proj output column slices (2048 / 512)
XGROUPS = [[0], [1], [2, 3], [4, 5], [6, 7], [8, 9, 10, 11], [12, 13, 14, 15]]

# Measured configs (HW exec / end-to-end relative error):
#   fp16/fp16: ~83.5us / 7.2e-4   bf16/bf16: ~86us / 5.8e-3
DT_BIG = "fp16"            # qkv + proj matmul operand dtype: fp16|f32r|bf16
DT_ATT = "fp16"            # attention matmul operand dtype:  fp16|f32r|bf16

_compiled = {}


def _dt(mybir, name):
    return {"f32r": mybir.dt.float32r, "bf16": mybir.dt.bfloat16,
            "fp16": mybir.dt.float16}[name]


def _build_program(dt_big, dt_att):
    import concourse.mybir as mybir
    import concourse.tile as tile
    from concourse import bacc

    F32 = mybir.dt.float32
    DTB = _dt(mybir, dt_big)
    DTA = _dt(mybir, dt_att)

    nc = bacc.Bacc("TRN2", target_bir_lowering=False, debug=False)

    # xw = [xT | wqkT | wvT] packed on the 2048-row contraction axis
    XWW = T + 2 * CPC + CPC
    xw = nc.dram_tensor("xw", [C, XWW], DTB, kind="ExternalInput")
    pw = nc.dram_tensor("pw", [CPC, C], DTB, kind="ExternalInput")
    maskA = nc.dram_tensor("maskA", [128, SW2], DTA, kind="ExternalInput")
    maskM = nc.dram_tensor("maskM", [128, SW2], DTA, kind="ExternalInput")
    out_d = nc.dram_tensor("out", [T, C], DTB, kind="ExternalOutput")

    with tile.TileContext(nc) as tc:
        with (
            tc.tile_pool(name="persist", bufs=1) as per,
            tc.tile_pool(name="work", bufs=8) as wk,
            tc.tile_pool(name="ps", bufs=2, space="PSUM") as pp,
        ):
            # ---- input loads first: xw in few large DMAs (group 0 split so
            # the first matmul chunk lands fast, weights before activations)
            xwt = []            # kc -> [128, XWW] view
            for gi, g in enumerate(XGROUPS):
                gw = len(g) * XWW
                gt = per.tile([128, gw], DTB, tag=f"xwg{g[0]}")
                src = xw[g[0] * 128:(g[0] + len(g)) * 128, :]
                # alternate the two HWDGE rings (ACT + sync queues) so the
                # ~0.8us-per-dma_start descriptor generation runs in
                # parallel and the first bytes land sooner
                eng = nc.scalar if gi % 2 == 0 else nc.sync
                eng.dma_start(
                    out=gt.rearrange("p (n c) -> p n c", n=len(g)),
                    in_=src.rearrange("(n p) c -> p n c", p=128))
                for i, kc in enumerate(g):
                    xwt.append(gt[:, i * XWW:(i + 1) * XWW])
            xt = [t[:, 0:T] for t in xwt]
            wqk_t = [t[:, T:T + 2 * CPC] for t in xwt]
            wv_t = [t[:, T + 2 * CPC:XWW] for t in xwt]

            # ---- small constants after xw (needed only mid-kernel) ----
            # masks/pw ride the scalar ring: keeps the sync ring free to
            # issue the remaining xw groups ~1.6us earlier (ACT itself is
            # idle until the attention phase, so the issue time is free)
            mA = per.tile([128, SW2], DTA, tag="mA")
            nc.scalar.dma_start(out=mA, in_=maskA[:, :])
            mM = per.tile([128, SW2], DTA, tag="mM")
            nc.scalar.dma_start(out=mM, in_=maskM[:, :])
            pw_t = []
            for k2 in range(2):
                t = per.tile([128, C], DTB, tag=f"pw{k2}")
                nc.scalar.dma_start(out=t, in_=pw[k2 * 128:(k2 + 1) * 128, :])
                pw_t.append(t)
            onecol = per.tile([128, 1], F32, tag="onecol")
            nc.vector.memset(onecol, 1.0)
            zpad = per.tile([128, TP - T], F32, tag="zpad")
            nc.vector.memset(zpad, 0.0)

            # warm-up matmuls on memset data (no DMA dependency): keeps the
            # PE busy while the first x/w chunks stream in, so HAM
            # un-throttles the clock early.
            wusrc = per.tile([128, 256], F32, tag="wusrc")
            nc.vector.memset(wusrc, 1.0)
            wusrc16 = per.tile([128, 128], DTA, tag="wusrc16")
            nc.vector.memset(wusrc16, 1.0)
            # 12 f32 N=256 matmuls ~= 10us at the cold clock: the PE stays
            # busy through the IRAM-fetch/preamble window (first input bytes
            # land ~12us in) so HAM un-throttles early and phase 1 starts
            # warm the moment its data arrives.
            wu = pp.tile([128, 256], F32, tag="mm", name="wu", bufs=3)
            for _ in range(12):
                nc.tensor.matmul(out=wu, lhsT=wusrc[:, 0:128], rhs=wusrc,
                                 start=True, stop=True)

            # ---- phase 1: q,k projection (weight-stationary) ----
            # qkT[mc] rows: mc 0,1 = q (heads 0,1 / 2,3); mc 2,3 = k
            # mc alternates innermost so consecutive matmuls load different
            # weights (LDWEIGHTS overlaps via the background weight buffer).
            qkT = []
            for mc4 in range(4):
                t = per.tile([128, TP], DTA, tag=f"qkT{mc4}", name=f"qkT{mc4}")
                nc.vector.tensor_copy(out=t[:, T:TP], in_=zpad)
                qkT.append(t)
            # ni inner: 8 qkps tiles live across all 8 banks, and each
            # xw chunk is consumed by 8 back-to-back matmuls -- the sweep
            # rate then matches the input-DMA arrival rate (no mid-phase
            # PE stall -> no HAM re-throttle).
            gtag = {(0, 0): "mm", (0, 1): "mm", (1, 0): "mm", (1, 1): "st",
                    (2, 0): "st", (2, 1): "st", (3, 0): "zt", (3, 1): "zt"}
            qps = {}
            for mc in range(4):
                for ni, (n0, nw) in enumerate(QKCH):
                    qps[(mc, ni)] = pp.tile(
                        [128, nw], F32, tag=gtag[(mc, ni)],
                        name=f"qkps{mc}_{ni}", bufs=3 if gtag[(mc, ni)] != "zt" else 2)
            for kc in range(KC):
                for ni, (n0, nw) in enumerate(QKCH):
                    for mc in range(4):
                        nc.tensor.matmul(
                            out=qps[(mc, ni)],
                            lhsT=wqk_t[kc][:, mc * 128:(mc + 1) * 128],
                            rhs=xt[kc][:, n0:n0 + nw],
                            start=(kc == 0), stop=(kc == KC - 1),
                        )
            for ni, (n0, nw) in enumerate(QKCH):
                for mc in range(4):
                    if (mc + ni) % 2 == 0:
                        nc.scalar.copy(out=qkT[mc][:, n0:n0 + nw],
                                       in_=qps[(mc, ni)])
                    else:
                        nc.vector.tensor_copy(out=qkT[mc][:, n0:n0 + nw],
                                              in_=qps[(mc, ni)])

            # ---- phase 2: v projection (x-stationary: v in (tokens, dims)) ----
            vone = {}  # (b, jc) -> [th, 4, 65] tile (per-head v cols + ones col)

            def emit_v(b, jc):
                t0, th = TOKCH[jc]
                vps = pp.tile([th, CPC], F32, tag="mm", name=f"vps{b}_{jc}", bufs=3)
                for kc in range(KC):
                    nc.tensor.matmul(
                        out=vps,
                        lhsT=xt[kc][:, b * N + t0: b * N + t0 + th],
                        rhs=wv_t[kc],
                        start=(kc == 0), stop=(kc == KC - 1),
                    )
                vt = per.tile([th, HPC, HD + 1], DTA, tag=f"vone{b}_{jc}",
                              name=f"vone{b}_{jc}")
                nc.vector.tensor_copy(
                    out=vt[:, :, 0:HD],
                    in_=vps.rearrange("t (h d) -> t h d", h=HPC))
                nc.vector.tensor_copy(
                    out=vt[:, :, HD],
                    in_=onecol[:th, 0:1].to_broadcast((th, HPC)))
                vone[(b, jc)] = vt



            # ---- phase 3 + 4: windowed attention, proj interleaved ----
            # per (b,h): ST packed [128, 272]; logits = (ST + A) * M
            # E = exp(logits); zT' = [v|1]^T @ E  (row HD = softmax denom)
            # After batch b completes: project its 197 columns (spread over
            # the next batch's head slots).
            YT = [per.tile([128, T], DTB, tag=f"YT{k2}", name=f"YT{k2}")
                  for k2 in range(2)]

            def attn_s(b, p):
                """score matmuls for head pair (2p, 2p+1) -> two st tiles.
                Each head gets its OWN tile: the two heads contract on
                different PE row groups (partitions 0:64 vs 64:128), and
                different-row-group matmuls draining into the same PSUM
                bank abort the device."""
                st = [pp.tile([128, SW], F32, tag="st",
                              name=f"st{b}_{p}_{s2}", bufs=3)
                      for s2 in range(2)]
                if b >= 2:
                    # keep-warm dummy: HAM re-throttles the PE clock to
                    # 1.2GHz when the late phase goes sparse; this is
                    # overwritten by the real start=True matmuls below
                    nc.tensor.matmul(out=st[0][:, 0:64],
                                     lhsT=wusrc16, rhs=wusrc16[:, 0:64],
                                     start=True, stop=True)
                # blk outer / head inner: adjacent matmuls use different PE
                # row groups AND different PSUM banks, so they execute
                # concurrently in the array (per-subarray concurrency)
                for (j0, jh, i0, iw, c0) in BLOCKS:
                    for s2 in range(2):
                        qTh = qkT[p][64 * s2:64 * s2 + 64, :]
                        kTh = qkT[2 + p][64 * s2:64 * s2 + 64, :]
                        nc.tensor.matmul(
                            out=st[s2][0:jh, c0:c0 + iw],
                            lhsT=kTh[:, b * N + j0: b * N + j0 + jh],
                            rhs=qTh[:, b * N + i0: b * N + i0 + iw],
                            start=True, stop=True,
                        )
                return st

            def attn_rest(b, p, st):
                """mask, exp, AV, normalize for head pair (2p, 2p+1).
                The per-head PSUM score tiles merge into one wide SBUF tile
                at the first (1x-rate PSUM-read) op; everything downstream
                runs once per pair. The AV matmuls are full-array, so the
                packed zt bank is safe."""
                sa = wk.tile([128, SW2], DTA, tag="sa")
                for s2 in range(2):
                    nc.vector.tensor_tensor(
                        sa[:, s2 * SW:(s2 + 1) * SW], st[s2], mA[:, 0:SW],
                        mybir.AluOpType.add)
                sm = wk.tile([128, SW2], DTA, tag="sm")
                nc.vector.tensor_tensor(sm, sa, mM, mybir.AluOpType.mult)
                e = wk.tile([128, SW2], DTA, tag="e")
                nc.scalar.activation(
                    out=e, in_=sm, func=mybir.ActivationFunctionType.Exp)
                zt = pp.tile([HD + 1, SW2], F32, tag="zt", name=f"zt{b}_{p}")
                first = True
                for s2 in range(2):
                    for blk, (j0, jh, i0, iw, c0) in enumerate(BLOCKS):
                        nc.tensor.matmul(
                            out=zt[:, s2 * SW + i0: s2 * SW + i0 + iw],
                            lhsT=vone[(b, blk)][0:jh, 2 * p + s2, :],
                            rhs=e[0:jh, s2 * SW + c0: s2 * SW + c0 + iw],
                            start=first, stop=(s2 == 1 and blk == 1),
                        )
                        first = False
                # stage only the denominator row to SBUF (the custom-DVE
                # reciprocal cannot read PSUM on HW); z is normalized
                # straight out of PSUM. Pad columns hold den=128 (mask
                # makes e=1 there) so the reciprocal stays finite.
                dent = wk.tile([1, SW2], F32, tag="dent")
                nc.scalar.copy(out=dent, in_=zt[HD:HD + 1, :])
                rrow = wk.tile([1, SW2], F32, tag="rrow")
                nc.vector.reciprocal_approx_fast(out=rrow, in_=dent)
                rb = wk.tile([64, SW2], F32, tag="rb")
                nc.gpsimd.partition_broadcast(rb, rrow)
                for s2 in range(2):
                    nc.vector.tensor_tensor(
                        YT[p][s2 * 64:(s2 + 1) * 64, b * N:(b + 1) * N],
                        zt[0:HD, s2 * SW:s2 * SW + N],
                        rb[:, s2 * SW:s2 * SW + N], mybir.AluOpType.mult)

            obt = {}
            for b2 in range(B):
                for jc2 in range(2):
                    obt[(b2, jc2)] = per.tile(
                        [PROJCH[jc2][1], C], DTB, tag=f"ob{jc2}",
                        name=f"ob{b2}_{jc2}", bufs=2)

            pj_pend = {}

            def proj_units(b, units, tag, k2s=(0, 1)):
                # units arrive as (jc, ns) pairs with even ns: each call
                # handles (ns, ns+1) with k2 outer so the YT weight tile is
                # reused by two consecutive matmuls. k2s allows splitting
                # the accumulation across two call sites (tail shortening:
                # the k2=0 half only needs YT[0] = head pair 0).
                for (jc, ns) in units:
                    t0, th = PROJCH[jc]
                    tsl = slice(b * N + t0, b * N + t0 + th)
                    if 0 in k2s:
                        pps = [pp.tile([th, 512], F32, tag="mm",
                                       name=f"pj{tag}_{jc}_{ns + u}", bufs=3)
                               for u in range(2)]
                        pj_pend[(b, jc, ns)] = pps
                        if (tag == "p3" or b >= 2) and (jc, ns) == units[0]:
                            nc.tensor.matmul(out=pps[0][0:th, 0:64],
                                             lhsT=wusrc16[:, 0:th],
                                             rhs=wusrc16[:, 0:64],
                                             start=True, stop=True)
                    else:
                        pps = pj_pend.pop((b, jc, ns))
                    for k2 in k2s:
                        for u in range(2):
                            nc.tensor.matmul(
                                out=pps[u],
                                lhsT=YT[k2][:, tsl],
                                rhs=pw_t[k2][:, (ns + u) * 512:(ns + u + 1) * 512],
                                start=(k2 == 0), stop=(k2 == 1),
                            )
                    if 1 not in k2s:
                        continue
                    ob = obt[(b, jc)]
                    for u in range(2):
                        # 3:1 toward ScalarE mid-kernel (VectorE carries the
                        # chain ops); 1:1 for the final batch where the
                        # evacuations are the critical path to the last DMA
                        if u == 1 and (jc == 1 or tag == "p3"):
                            nc.vector.tensor_copy(
                                out=ob[:, (ns + u) * 512:(ns + u + 1) * 512],
                                in_=pps[u])
                        else:
                            nc.scalar.copy(
                                out=ob[:, (ns + u) * 512:(ns + u + 1) * 512],
                                in_=pps[u])
                    csl = slice(ns * 512, (ns + 2) * 512)
                    nc.sync.dma_start(out=out_d[tsl, csl], in_=ob[:, csl])

            # software-pipeline: emit S matmuls one (b,h) ahead so the PE has
            # independent work while the previous chain's DVE/ACT stages run.
            # Fillers per head slot keep the PE dense: v-projections for the
            # next batches, and the previous batch's proj M-chunks.
            PUNITS = [(jc, ns) for jc in range(2) for ns in range(0, NS, 2)]
            bps = [(b, p) for b in range(B) for p in range(2)]
            vfill = [(1, 0), (1, 1), (2, 0), (2, 1), (3, 0), (3, 1)]
            # first chain's scores before the v matmuls: its mask/exp stages
            # then overlap the v projection on the PE
            sts = {bps[0]: attn_s(*bps[0])}
            emit_v(0, 0)
            emit_v(0, 1)
            for idx, (b, p) in enumerate(bps):
                if idx + 1 < len(bps):
                    sts[bps[idx + 1]] = attn_s(*bps[idx + 1])
                attn_rest(b, p, sts.pop((b, p)))
                if vfill:
                    emit_v(*vfill.pop(0))
                if b >= 1:
                    proj_units(b - 1, PUNITS[p * 2:(p + 1) * 2], f"p{b-1}")
            proj_units(3, PUNITS, "p3")

    nc.compile()
    return nc


def _host_masks(np_att):
    i = np.arange(N)[:, None]
    j = np.arange(N)[None, :]
    d = np.abs(i - j).astype(np.float32)
    in_win = (j >= i - WIN) & (j < i + WIN)
    m = np.where(in_win, (WIN - d / 2.0) / WIN, 0.0).astype(np.float32)
    # transposed (j on rows): logits[j,i] = (ST[j,i] + A[j,i]) * M[j,i]
    multT = np.where(in_win, m * SCALE, 1.0).astype(np.float32).T
    addT = np.where(in_win, 0.0, NEG).astype(np.float32).T
    # pack the two banded blocks side by side into [128, SW] tiles
    mult = np.zeros((128, SW), dtype=np.float32)
    addm = np.zeros((128, SW), dtype=np.float32)
    for blk, (j0, jh, i0, iw, c0) in enumerate(BLOCKS):
        iw_r = min(iw, N - i0)  # data columns (rest stays pad)
        mult[0:jh, c0:c0 + iw_r] = multT[j0:j0 + jh, i0:i0 + iw_r]
        addm[0:jh, c0:c0 + iw_r] = addT[j0:j0 + jh, i0:i0 + iw_r]
    # duplicate for the two heads packed side by side; pad cells stay
    # (M=0, A=0) so e = exp(0) = 1 and denominators remain finite.
    return (np.tile(addm, (1, 2)).astype(np_att),
            np.tile(mult, (1, 2)).astype(np_att))


def _np_dt(name):
    if name == "bf16":
        import ml_dtypes
        return ml_dtypes.bfloat16
    if name == "fp16":
        return np.float16
    return np.float32


def _make_in_maps(x, qkv_w, proj_w):
    npb = _np_dt(DT_BIG)
    npa = _np_dt(DT_ATT)
    xT = x.reshape(T, C).T
    addm, mult = _host_masks(npa)
    in_maps = []
    for d in range(NCORES):
        r = slice(d * CPC, (d + 1) * CPC)
        wqk_d = np.concatenate(
            [qkv_w[r, :], qkv_w[C + d * CPC: C + (d + 1) * CPC, :]], axis=0).T
        wv_d = qkv_w[2 * C + d * CPC: 2 * C + (d + 1) * CPC, :].T
        xw_d = np.ascontiguousarray(
            np.concatenate([xT, wqk_d, wv_d], axis=1)).astype(npb)
        pw_d = np.ascontiguousarray(proj_w[:, r].T).astype(npb)
        in_maps.append({"xw": xw_d, "pw": pw_d, "maskA": addm, "maskM": mult})
    return in_maps


def kernel(x, qkv_w, proj_w, proj_b):
    from concourse.bass_utils import run_bass_kernel_spmd

    key = (DT_BIG, DT_ATT)
    if key not in _compiled:
        _compiled[key] = _build_program(*key)
    nc = _compiled[key]

    x = np.asarray(x, dtype=np.float32)
    qkv_w = np.asarray(qkv_w, dtype=np.float32)
    proj_w = np.asarray(proj_w, dtype=np.float32)
    proj_b = np.asarray(proj_b, dtype=np.float32)

    in_maps = _make_in_maps(x, qkv_w, proj_w)
    res = run_bass_kernel_spmd(nc, in_maps, core_ids=list(range(NCORES)))
    acc = np.zeros((T, C), dtype=np.float32)
    for r in res.results:
        acc += r["out"].astype(np.float32)
    out = acc + proj_b[None, :]
    return np.ascontiguousarray(out).reshape(B, N, C)
